# revision 21
# baseline (speedup 1.0000x reference)
"""Trainium2 Bass kernel for nn_CrossAttention (B=2, N=1024, L=4096, C=1024, H=16).

Sharding: head-parallel across 8 NeuronCores (2 heads per core).
Each core computes q/k/v projections for its 2 heads (f32r matmuls), the full
NxL attention for those heads, and a partial output projection over its 128
head-dims; a per-batch ReduceScatter sums the partials and leaves each core
with its 128-row slice of the final output.
"""

import functools

import numpy as np

B, N, L, C = 2, 1024, 4096, 1024
H, D = 16, 64
SCALE = D ** -0.5
NCORES = 8
LOCD = C // NCORES       # 128 local head-dims per core (2 heads x 64)
R = B * N                # 2048 query rows
RL = B * L               # 8192 key rows
ROWS_PER_CORE_B = N // NCORES  # 128 output rows per core per batch


def _split_excess_waits(nc, max_waits=1):
    """walrus in this container rejects >1 sync wait per instruction; hoist
    excess waits onto NoOps inserted before the offender on the same engine."""
    import concourse.mybir as mybir

    ctr = 0
    for fn in nc.m.functions:
        for blk in fn.blocks:
            insts = list(blk.instructions)
            new_insts = []
            changed = False
            for ins in insts:
                si = getattr(ins, "sync_info", None)
                if si is not None and si.on_wait and len(si.on_wait) > max_waits:
                    waits = list(si.on_wait)
                    excess, keep = waits[:-max_waits], waits[-max_waits:]
                    for i in range(0, len(excess), max_waits):
                        ctr += 1
                        nop = mybir.InstNoOp(
                            name=f"waitsplit_{ctr}",
                            engine=ins.engine,
                            sync_info=mybir.SyncInfo(
                                on_wait=excess[i : i + max_waits], on_update=[]
                            ),
                            text_hint="waitsplit",
                        )
                        new_insts.append(nop)
                        nc.register_instruction(nop, overwrite=True)
                    ins.sync_info = mybir.SyncInfo(
                        on_wait=keep, on_update=list(si.on_update)
                    )
                    changed = True
                new_insts.append(ins)
            if changed:
                blk.instructions = new_insts


@functools.cache
def _build():
    import concourse.bass as bass
    import concourse.mybir as mybir
    import concourse.tile as tile
    from concourse.tile import add_dep_helper

    f32 = mybir.dt.float32
    f32r = mybir.dt.float32r

    nc = bass.Bass()

    # ---- DRAM parameters (per-core views prepared on host) ----
    xT = nc.declare_dram_parameter("xT", [C, R], f32r, isOutput=False)
    yT = nc.declare_dram_parameter("yT", [C, RL], f32r, isOutput=False)
    wqT = nc.declare_dram_parameter("wqT", [C, LOCD], f32r, isOutput=False)
    wkT = nc.declare_dram_parameter("wkT", [C, LOCD], f32r, isOutput=False)
    wvT = nc.declare_dram_parameter("wvT", [C, LOCD], f32r, isOutput=False)
    wplT = nc.declare_dram_parameter("wplT", [LOCD, C], f32r, isOutput=False)
    identm = nc.declare_dram_parameter("identm", [128, 128], f32r, isOutput=False)
    onesm = nc.declare_dram_parameter("onesm", [128, 128], f32r, isOutput=False)
    out_partial = nc.declare_dram_parameter("out_partial", [R, C], f32, isOutput=True)

    core_ids = list(range(NCORES))
    KT = C // 128  # 8 contraction tiles

    # rearranged DRAM views: [(kt p), cols] -> [p, kt, cols]
    xTr = xT.rearrange("(kt p) c -> p kt c", p=128)
    yTr = yT.rearrange("(kt p) c -> p kt c", p=128)

    with tile.TileContext(nc) as tc:
        with (
            tc.tile_pool(name="const", bufs=1) as constp,
            tc.tile_pool(name="weights", bufs=1) as wpool,
            tc.tile_pool(name="standing", bufs=1) as stand,
            tc.tile_pool(name="yx", bufs=3) as ypool,
            tc.tile_pool(name="vtsb", bufs=2) as vtpool,
            tc.tile_pool(name="pt", bufs=3) as ptpool,
            tc.tile_pool(name="small", bufs=2) as smallp,
            tc.tile_pool(name="drain", bufs=3) as drainp,
            tc.tile_pool(name="psG", bufs=1, space="PSUM") as psG,
            tc.tile_pool(name="psS", bufs=2, space="PSUM") as psS,
            tc.tile_pool(name="psV", bufs=1, space="PSUM") as psV,
        ):
            # ---- constants / weights ----
            ident = constp.tile([128, 128], f32r)
            nc.gpsimd.dma_start(ident[:], identm[:])
            ones1 = constp.tile([1, 64], f32r)
            nc.gpsimd.dma_start(ones1[:], onesm[0:1, 0:64])
            wpl_s = constp.tile([LOCD, C], f32r)
            nc.gpsimd.dma_start(wpl_s[:], wplT[:])

            wq_s = wpool.tile([128, KT, LOCD], f32r, tag="wq")
            wk_s = wpool.tile([128, KT, LOCD], f32r, tag="wk")
            wv_s = wpool.tile([128, KT, LOCD], f32r, tag="wv")
            nc.gpsimd.dma_start(wq_s[:], wqT.rearrange("(kt p) m -> p kt m", p=128))
            nc.gpsimd.dma_start(wk_s[:], wkT.rearrange("(kt p) m -> p kt m", p=128))
            nc.gpsimd.dma_start(wv_s[:], wvT.rearrange("(kt p) m -> p kt m", p=128))

            # ---- standing tensors ----
            qT_s = stand.tile([128, R], f32r, tag="qT")          # [locdim, (b,n)]
            kT_s = stand.tile([128, RL], f32r, tag="kT")         # [locdim, (b,l)]
            v_s = stand.tile([128, RL // 128, 130], f32r, tag="v")  # [l%128, LT, 130]
            ahat_s = stand.tile([128, R], f32r, tag="ahat")      # [locdim, (b,n)]
            # ones columns of v_aug (cols 64 and 129), filled from host ones
            ones_cols = v_s[:, :, 0:130].rearrange("p t (a c) -> p t a c", a=2, c=65)[
                :, :, :, 64:65
            ]
            ones_sb = constp.tile([128, 128], f32r)
            nc.gpsimd.dma_start(ones_sb[:], onesm[:])
            nc.vector.tensor_copy(
                out=ones_cols,
                in_=ones_sb[:].rearrange("p (t a one) -> p t a one", t=64, a=2, one=1),
            )

            # ---- phase 1a: qT projection, 512-col units ----
            for u in range(R // 512):
                src = ypool.tile([128, KT, 512], f32r, tag="yx", name=f"xsrc{u}")
                nc.sync.dma_start(src[:], xTr[:, :, u * 512 : (u + 1) * 512])
                acc = psG.tile([128, 2, 512], f32, tag="kv", name=f"qacc{u}")
                for kt in range(KT):
                    nc.tensor.matmul(
                        acc[:, 0, :],
                        lhsT=(wq_s[:, kt, :]),
                        rhs=(src[:, kt, :]),
                        start=(kt == 0),
                        stop=(kt == KT - 1),
                    )
                nc.vector.tensor_copy(
                    out=qT_s[:, u * 512 : (u + 1) * 512], in_=acc[:, 0, :]
                )

            # ---- phase 1b: kT + vT projections + v transpose, 512-col units ----
            for b in range(B):
                for u in range(L // 512):
                    off = b * L + u * 512
                    src = ypool.tile([128, KT, 512], f32r, tag="yx", name=f"ysrc{b}_{u}")
                    nc.sync.dma_start(src[:], yTr[:, :, off : off + 512])
                    acc = psG.tile([128, 2, 512], f32, tag="kv", name=f"kvacc{b}_{u}")
                    for kt in range(KT):
                        nc.tensor.matmul(
                            acc[:, 0, :],
                            lhsT=(wk_s[:, kt, :]),
                            rhs=(src[:, kt, :]),
                            start=(kt == 0),
                            stop=(kt == KT - 1),
                        )
                        nc.tensor.matmul(
                            acc[:, 1, :],
                            lhsT=(wv_s[:, kt, :]),
                            rhs=(src[:, kt, :]),
                            start=(kt == 0),
                            stop=(kt == KT - 1),
                        )
                    nc.scalar.copy(out=kT_s[:, off : off + 512], in_=acc[:, 0, :])
                    vt_sb = vtpool.tile([128, 512], f32r, tag="vtsb", name=f"vt{b}_{u}")
                    nc.scalar.copy(out=vt_sb[:], in_=acc[:, 1, :])
                    # transpose vT -> v (natural layout) via PE, 128x128 blocks
                    for j in range(4):
                        LT = off // 128 + j
                        vtr = psG.tile(
                            [128, 2, 512], f32, tag="kv", name=f"vtr{b}_{u}_{j}"
                        )
                        nc.tensor.transpose(
                            vtr[:, 0, 0:128].bitcast(f32r),
                            vt_sb[:, j * 128 : (j + 1) * 128],
                            ident[:],
                        )
                        nc.vector.tensor_copy(
                            out=v_s[:, LT, 0:130].rearrange(
                                "p (a c) -> p a c", a=2, c=65
                            )[:, :, 0:64],
                            in_=vtr[:, 0, 0:128]
                            .bitcast(f32r)
                            .rearrange("p (a c) -> p a c", a=2, c=64),
                        )

            # ---- phase 2: attention per batch, fused scores->exp->AV ----
            # ---- phase 3: partial out-projection + ReduceScatter per batch ----
            for b in range(B):
                for nc2 in range(2):
                    ncol = b * N + nc2 * 512
                    av = psV.tile([128, 2, 512], f32, tag="av", name=f"av{b}_{nc2}")
                    for lt in range(32):
                        koff = b * L + lt * 128
                        st = psS.tile([128, 2, 512], f32, tag="st", name=f"st{b}_{nc2}_{lt}")
                        pt = ptpool.tile([128, 2, 512], f32r, tag="pt", name=f"pt{b}_{nc2}_{lt}")
                        for h in range(2):
                            nc.tensor.matmul(
                                st[:, h, :],
                                lhsT=(kT_s[h * 64 : (h + 1) * 64, koff : koff + 128]),
                                rhs=(qT_s[h * 64 : (h + 1) * 64, ncol : ncol + 512]),
                                start=True,
                                stop=True,
                            )
                        nc.scalar.activation(
                            pt[:], st[:], mybir.ActivationFunctionType.Exp, scale=SCALE
                        )
                        for h in range(2):
                            nc.tensor.matmul(
                                av[0:65, h, :],
                                lhsT=(v_s[:, b * 32 + lt, h * 65 : h * 65 + 65]),
                                rhs=(pt[:, h, :]),
                                start=(lt == 0),
                                stop=(lt == 31),
                            )
                    for h in range(2):
                        recip = smallp.tile([1, 512], f32r, tag="recip", name=f"rc{b}_{nc2}_{h}")
                        with nc.allow_low_precision(
                            reason="f32r reciprocal feeds f32r broadcast matmul"
                        ):
                            nc.vector.reciprocal(recip[:], av[64:65, h, :])
                        bc_ps = psG.tile([128, 2, 512], f32, tag="kv", name=f"bp{b}_{nc2}_{h}")
                        nc.tensor.matmul(
                            bc_ps[0:64, 0, :],
                            lhsT=ones1[:],
                            rhs=recip[:],
                            start=True,
                            stop=True,
                        )
                        bcst = smallp.tile([64, 512], f32, tag="bcst", name=f"bc{b}_{nc2}_{h}")
                        nc.vector.tensor_copy(out=bcst[:], in_=bc_ps[0:64, 0, :])
                        nc.vector.tensor_mul(
                            out=ahat_s[h * 64 : (h + 1) * 64, ncol : ncol + 512],
                            in0=av[0:64, h, :],
                            in1=bcst[:],
                        )

                # partial projection for batch b: out_partial = Ahat_loc^T @ WpT_loc
                for m in range(8):
                    p_ps = psG.tile([128, 2, 512], f32, tag="kv", name=f"pp{b}_{m}")
                    for cb in range(2):
                        nc.tensor.matmul(
                            p_ps[:, cb, :],
                            lhsT=(ahat_s[:, b * N + m * 128 : b * N + (m + 1) * 128]),
                            rhs=(wpl_s[:, cb * 512 : (cb + 1) * 512]),
                            start=True,
                            stop=True,
                        )
                    part = drainp.tile([128, 1024], f32, tag="part", name=f"part{b}_{m}")
                    nc.vector.tensor_copy(
                        out=part[:].rearrange("p (a c) -> p a c", a=2, c=512),
                        in_=p_ps[:],
                    )
                    nc.scalar.dma_start(
                        out_partial[b * N + m * 128 : b * N + (m + 1) * 128, :],
                        part[:],
                    )

    _split_excess_waits(nc)
    return nc


def _prep_inputs(x, y, Wq, Wk, Wv, Wp, bp):
    x = np.ascontiguousarray(x, dtype=np.float32)
    y = np.ascontiguousarray(y, dtype=np.float32)
    xT = np.ascontiguousarray(x.reshape(R, C).T)
    yT = np.ascontiguousarray(y.reshape(RL, C).T)
    WpT = np.ascontiguousarray(np.asarray(Wp, np.float32).T)
    in_maps = []
    for i in range(NCORES):
        sl = slice(i * LOCD, (i + 1) * LOCD)
        in_maps.append(
            {
                "xT": xT,
                "yT": yT,
                "wqT": np.ascontiguousarray(np.asarray(Wq, np.float32)[sl, :].T),
                "wkT": np.ascontiguousarray(np.asarray(Wk, np.float32)[sl, :].T),
                "wvT": np.ascontiguousarray(np.asarray(Wv, np.float32)[sl, :].T),
                "wplT": np.ascontiguousarray(WpT[sl, :]),
                "identm": np.eye(128, dtype=np.float32),
                "onesm": np.ones((128, 128), dtype=np.float32),
            }
        )
    return in_maps


ROWS_B = R // NCORES  # 256 rows per core in the reduce launch


@functools.cache
def _build_reduce():
    import concourse.bass as bass
    import concourse.mybir as mybir
    import concourse.tile as tile

    f32 = mybir.dt.float32
    nc = bass.Bass()
    pstack = nc.declare_dram_parameter("pstack", [NCORES, ROWS_B, C], f32, isOutput=False)
    biasb = nc.declare_dram_parameter("biasb", [128, C], f32, isOutput=False)
    out_shard = nc.declare_dram_parameter("out_shard", [ROWS_B, C], f32, isOutput=True)

    with tile.TileContext(nc) as tc:
        with (
            tc.tile_pool(name="cn", bufs=1) as constp,
            tc.tile_pool(name="sb", bufs=4) as pool,
        ):
            bias_s = constp.tile([128, C], f32)
            nc.gpsimd.dma_start(bias_s[:], biasb[:])
            for t in range(ROWS_B // 128):
                acc = pool.tile([128, C], f32, tag="acc", name=f"acc{t}")
                nc.sync.dma_start(acc[:], pstack[0, t * 128 : (t + 1) * 128, :])
                for c in range(1, NCORES):
                    tmp = pool.tile([128, C], f32, tag="tmp", name=f"tmp{t}_{c}")
                    nc.sync.dma_start(tmp[:], pstack[c, t * 128 : (t + 1) * 128, :])
                    nc.vector.tensor_add(out=acc[:], in0=acc[:], in1=tmp[:])
                nc.vector.tensor_add(out=acc[:], in0=acc[:], in1=bias_s[:])
                nc.sync.dma_start(out_shard[t * 128 : (t + 1) * 128, :], acc[:])

    _split_excess_waits(nc)
    return nc


def kernel(x, y, Wq, Wk, Wv, Wp, bp):
    from concourse.bass_utils import run_bass_kernel_spmd

    nc = _build()
    in_maps = _prep_inputs(x, y, Wq, Wk, Wv, Wp, bp)
    res = run_bass_kernel_spmd(nc, in_maps, list(range(NCORES)))
    partials = [res.results[i]["out_partial"] for i in range(NCORES)]  # [R, C] each

    nc2 = _build_reduce()
    bias = np.ascontiguousarray(
        np.broadcast_to(np.asarray(bp, np.float32), (128, C))
    )
    in_maps2 = [
        {
            "pstack": np.ascontiguousarray(
                np.stack([p[j * ROWS_B : (j + 1) * ROWS_B, :] for p in partials])
            ),
            "biasb": bias,
        }
        for j in range(NCORES)
    ]
    res2 = run_bass_kernel_spmd(nc2, in_maps2, list(range(NCORES)))
    out = np.concatenate(
        [res2.results[j]["out_shard"] for j in range(NCORES)], axis=0
    )
    return out.reshape(B, N, C)
